# revision 27
# baseline (speedup 1.0000x reference)
"""GNN message-passing net on 8 Trainium2 cores.

Reference: x:[256,784,1] -> h1 = elu(spmm(x)@W1+b1) -> h2 = elu(spmm(h1)@W2+b2)
-> flat[B, N*C] -> relu(flat@Wf1+bf1) -> softmax(z@Wf2+bf2).

Strategy (all matmul operands bf16, fp32 PSUM accumulation):
  * Densify the sparse filter A (784x784, ~1% nz) on the host; spmm becomes
    dense matmuls on the PE array.
  * F=1 makes conv1 an outer product: out1 = A @ X^T [784,256] shared by all
    channels; h1_c = elu(W1[c]*out1+b1[c]) with per-channel big-tile elu
    (elu(t)=min(exp(t),1)+relu(t)-1) over o1 staged in SBUF.
  * Conv2 spmm channel-sharded: core k computes out2_c = A @ h1_c for
    channels 4k..4k+3, full batch.  The channel->node AllToAll is split into
    FOUR node-group quarters: quarter q carries all 32 channels for nodes
    [112j+28q, 112j+28q+28) of each destination j, so the mix + elu + fc1
    work for quarter q overlaps the transfer of quarter q+1.  A tiny dummy
    collective issued at t~0 absorbs the one-time cc-stream barrier.
  * Received quarter layout packs (s%4, channel) into 128 partitions so the
    kron(I4, W2) mix matmul and the elementwise elu run at full width.
  * FC1 K-sharded with z in [batch, h] layout (F=512 matmuls); each core
    adds bf1/8 before a 2-way (h-halved) bf16 ReduceScatter over batch
    blocks, packed/unpacked with single strided DMAs.
  * Tail: relu, PE-transpose of z, FC2 (+bf2 via ones-row matmul), softmax
    without max-subtraction (logits are O(4)), Exp with fused row-sum.
"""
import json

import numpy as np

import concourse.bass as bass
import concourse.mybir as mybir
import concourse.tile as tile
from concourse.bass_utils import run_bass_kernel_spmd

B, N, F, E = 256, 784, 1, 6272
C, H, N_OUT = 32, 512, 10
NCORE = 8
CPC = C // NCORE      # 4 channels per core in conv2
P = 112               # 784 = 7 * 112
KN = N // P           # 7 node chunks
NPAD = P * NCORE      # 896 padded nodes for the node reshard
NG = 4                # node-group quarters per core block
NS = P // NG          # 28 nodes per quarter
SH = NS // 4          # 7 sh chunks per quarter (s = sh*4 + s4)
BPC = B // NCORE      # 32 batch rows per core
HJ = H // 128         # 4 h chunks

f32 = mybir.dt.float32
bf16 = mybir.dt.bfloat16
AF = mybir.ActivationFunctionType
ALU = mybir.AluOpType
AX = mybir.AxisListType


# ---------------------------------------------------------------------------
# BIR post-pass: this walrus build rejects instructions with >1 sync-wait;
# split extras onto standalone EventSemaphore instructions (same engine,
# inserted just before, so the engine stream stalls identically).
def _split_waits(bir: dict, max_waits: int = 1) -> dict:
    n = [0]
    for fn in bir.get("functions", []):
        for blk in fn.get("blocks", []):
            out = []
            for ins in blk.get("instructions", []):
                si = ins.get("sync_info") or {}
                waits = si.get("on_wait") or []
                if len(waits) > max_waits:
                    for w in waits[max_waits:]:
                        n[0] += 1
                        out.append({
                            "name": f"I-waitsplit-{n[0]}",
                            "opcode": "EventSemaphore",
                            "engine": ins["engine"],
                            "ins": [], "outs": [],
                            **({"debug": ins["debug"]} if "debug" in ins else {}),
                            "sync_info": {"on_update": [], "on_wait": [w]},
                        })
                    si = dict(si)
                    si["on_wait"] = waits[:max_waits]
                    ins = dict(ins)
                    ins["sync_info"] = si
                out.append(ins)
            blk["instructions"] = out
    return bir


def _install_wait_splitter(nc):
    orig = nc.to_json_bytes
    nc.to_json_bytes = lambda: json.dumps(_split_waits(json.loads(orig()))).encode()


# ---------------------------------------------------------------------------
def _build_program():
    nc = bass.Bass(num_devices=NCORE)

    at_d = nc.dram_tensor("at", [P, KN * N], bf16, kind="ExternalInput")
    xt_d = nc.dram_tensor("xt", [P, KN * B], bf16, kind="ExternalInput")
    wf1_d = nc.dram_tensor("wf1", [NS * 128, H], bf16, kind="ExternalInput")
    wb_d = nc.dram_tensor("wb", [1, 2 * CPC], f32, kind="ExternalInput")
    w2k_d = nc.dram_tensor("w2k", [128, 128], bf16, kind="ExternalInput")
    b2k_d = nc.dram_tensor("b2k", [128, 1], f32, kind="ExternalInput")
    bf18_d = nc.dram_tensor("bf18", [1, H], f32, kind="ExternalInput")
    wf2_d = nc.dram_tensor("wf2", [128, HJ * N_OUT], bf16, kind="ExternalInput")
    bf2_d = nc.dram_tensor("bf2", [1, N_OUT], bf16, kind="ExternalInput")
    id_d = nc.dram_tensor("idm", [BPC, BPC], f32, kind="ExternalInput")
    out_d = nc.dram_tensor("out", [BPC, N_OUT], f32, kind="ExternalOutput")

    with tile.TileContext(nc) as tc:
        with (
            tc.tile_pool(name="big", bufs=1) as big,
            tc.tile_pool(name="small", bufs=1) as small,
            tc.tile_pool(name="ework", bufs=3) as ework,
            tc.tile_pool(name="rwork", bufs=3) as rwork,
            tc.tile_pool(name="ps1", bufs=2, space="PSUM") as ps1,
            tc.tile_pool(name="ps2", bufs=2, space="PSUM") as ps2,
            tc.tile_pool(name="psm", bufs=2, space="PSUM") as psm,
            tc.tile_pool(name="psf", bufs=2, space="PSUM") as psf,
            tc.tile_pool(name="dram", bufs=1, space="DRAM") as dram,
        ):
            # ---- resident inputs ------------------------------------------
            at_sb = big.tile([P, KN, N], bf16)
            xt_sb = big.tile([P, KN, B], bf16)
            wf1_sb = big.tile([128, NS, H], bf16)
            wb_sb = small.tile([1, 2 * CPC], f32)
            w2k_sb = small.tile([128, 128], bf16)
            b2k_sb = small.tile([128, 1], f32)
            bf18_sb = small.tile([1, H], f32)
            wf2_sb = small.tile([128, HJ, N_OUT], bf16)
            bf2_sb = small.tile([1, N_OUT], bf16)
            id_sb = small.tile([BPC, BPC], f32)
            ones = small.tile([1, 128], bf16)
            ones_f = small.tile([1, 128], f32)

            xt_ap = xt_d[:].rearrange("p (k b) -> p k b", k=KN)
            at_ap = at_d[:].rearrange("p (k n) -> p k n", k=KN)
            # sync queue order = availability order for conv1 startup
            nc.sync.dma_start(xt_sb[:, 0:1, :], xt_ap[:, 0:1, :])
            nc.sync.dma_start(at_sb[:, 0:1, :], at_ap[:, 0:1, :])
            nc.sync.dma_start(wb_sb[:], wb_d[:])
            for kc in range(1, KN):
                nc.sync.dma_start(xt_sb[:, kc:kc + 1, :], xt_ap[:, kc:kc + 1, :])
                nc.sync.dma_start(at_sb[:, kc:kc + 1, :], at_ap[:, kc:kc + 1, :])
            nc.sync.dma_start(w2k_sb[:], w2k_d[:])
            nc.sync.dma_start(b2k_sb[:], b2k_d[:])
            nc.sync.dma_start(bf18_sb[:], bf18_d[:])
            nc.sync.dma_start(wf2_sb[:], wf2_d[:].rearrange("p (j o) -> p j o", j=HJ))
            nc.sync.dma_start(bf2_sb[:], bf2_d[:])
            nc.sync.dma_start(id_sb[:], id_d[:])
            nc.sync.dma_start(wf1_sb[:],
                              wf1_d[:].rearrange("(s p) h -> p s h", p=128))
            nc.vector.memset(ones[:], 1.0)
            nc.vector.memset(ones_f[:], 1.0)

            # ---- broadcast W1/b1 channel scalars across partitions --------
            ps_bc = ps1.tile([128, 2 * CPC], f32, tag="ps1")
            nc.tensor.matmul(ps_bc[:], ones_f[0:1, 0:128], wb_sb[:])
            wband = small.tile([128, 2 * CPC], f32)
            nc.vector.tensor_copy(wband[:], ps_bc[:])

            # ---- conv1: o1 = A @ X^T staged to SBUF (bf16: 2x ACT/DVE) ----
            o1s = big.tile([P, KN, B], bf16)
            for mc in range(KN):
                o1 = ps1.tile([P, B], f32, tag="ps1")
                for kc in range(KN):
                    nc.tensor.matmul(
                        o1[:],
                        at_sb[:, kc, mc * P:(mc + 1) * P],
                        xt_sb[:, kc, :],
                        start=(kc == 0), stop=(kc == KN - 1),
                    )
                if mc % 2 == 0:
                    nc.scalar.copy(o1s[:, mc, :], o1[:])
                else:
                    nc.vector.tensor_copy(o1s[:, mc, :], o1[:])

            # bf1/8 broadcast across partitions (needed only at fc1 time)
            ps_bf = psm.tile([128, H], f32, tag="psm")
            nc.tensor.matmul(ps_bf[:], ones_f[0:1, 0:128], bf18_sb[:])
            bf1b = small.tile([128, H], f32)
            nc.vector.tensor_copy(bf1b[:], ps_bf[:])

            # ---- conv1 elu: h1_c = elu(w1_c * o1 + b1_c), big tiles -------
            h1_sb = big.tile([P, CPC, KN, B], bf16)
            for c, k0, k1 in ((0, 0, 4), (1, 0, 4), (2, 0, 4), (3, 0, 4),
                              (0, 4, KN), (1, 4, KN), (2, 4, KN), (3, 4, KN)):
                fw = (k1 - k0) * B
                sc = wband[0:P, c:c + 1]
                bi = wband[0:P, CPC + c:CPC + c + 1]
                e = ework.tile([P, 4 * B], bf16)
                nc.scalar.activation(e[:, 0:fw], o1s[:, k0:k1, :], AF.Exp,
                                     bias=bi, scale=sc)
                r = rwork.tile([P, 4 * B], bf16)
                nc.scalar.activation(r[:, 0:fw], o1s[:, k0:k1, :], AF.Relu,
                                     bias=bi, scale=sc)
                nc.vector.tensor_scalar(e[:, 0:fw], e[:, 0:fw],
                                        1.0, -1.0, ALU.min, ALU.add)
                nc.vector.tensor_tensor(h1_sb[:, c, k0:k1, :], e[:, 0:fw],
                                        r[:, 0:fw], ALU.add)

            # ---- conv2 spmm (batch-half major) + AllToAll per half --------
            # half bh block j carries [(cl, s), b128] = my 4 channels for
            # nodes 112j + s, batches bh*128+...; receiver repacks to
            # [(ng, c), s, b128].  cc(bh0) fires right after half 0 of
            # conv2, overlapping conv2 half 1; mix/fc1 of half 0 overlap
            # cc(bh1).
            out2_sb = big.tile([P, CPC, KN, B], bf16)
            zpad = small.tile([P, CPC, 128], bf16)
            nc.vector.memset(zpad[:], 0.0)
            a2a_in = [dram.tile([NCORE, CPC * P, 128], bf16, name=f"a2ai{i}")
                      for i in range(2)]
            a2a_out = [dram.tile([NCORE, CPC * P, 128], bf16, name=f"a2ao{i}")
                       for i in range(2)]
            r_sb = [big.tile([128, NS, 128], bf16, name=f"rsb{i}")
                    for i in range(2)]
            for bh in range(2):
                nc.gpsimd.dma_start(
                    a2a_in[bh][KN].rearrange("(cl p) b -> p cl b", cl=CPC),
                    zpad[:])
            for bh in range(2):
                b0 = bh * 128
                for mo in range(KN):
                    o2 = ps2.tile([P, CPC, 128], f32, tag="ps2")
                    for kc in range(KN):
                        nc.tensor.matmul(
                            o2[:],
                            at_sb[:, kc, mo * P:(mo + 1) * P],
                            h1_sb[:, :, kc, b0:b0 + 128],
                            start=(kc == 0), stop=(kc == KN - 1),
                        )
                    if mo % 2 == 0:
                        nc.vector.tensor_copy(
                            out2_sb[:, :, mo, b0:b0 + 128], o2[:])
                    else:
                        nc.scalar.copy(out2_sb[:, :, mo, b0:b0 + 128], o2[:])
                    nc.gpsimd.dma_start(
                        a2a_in[bh][mo].rearrange("(cl p) b -> p cl b",
                                                 cl=CPC),
                        out2_sb[:, :, mo, b0:b0 + 128])
                nc.gpsimd.collective_compute(
                    "AllToAll", ALU.bypass,
                    replica_groups=[list(range(NCORE))],
                    ins=[a2a_in[bh].opt()], outs=[a2a_out[bh].opt()],
                )
                src = a2a_out[bh][:].rearrange(
                    "k (cl g s) b -> g (k cl) s b", cl=CPC, g=NG)
                for ng in range(NG):
                    nc.sync.dma_start(r_sb[bh][ng * 32:(ng + 1) * 32, :, :],
                                      src[ng])

            # ---- mix + elu + fc1, pipelined per 4-node chunk --------------
            h2_sb = big.tile([128, NS, B], bf16)
            zsb = small.tile([128, 2, H], bf16)
            rs_in = dram.tile([NCORE, BPC, H], bf16)
            rsd = rs_in[:].rearrange("(jh jl) b h -> jh (jl b) h", jh=2)
            zps = [psf.tile([128, H], f32, tag="psf", name=f"zp{i}")
                   for i in range(2)]

            def fc1_mms(bh, t):
                nc.tensor.matmul(
                    zps[bh][:],
                    h2_sb[:, t, bh * 128:(bh + 1) * 128],
                    wf1_sb[:, t, :],
                    start=(t == 0), stop=(t == NS - 1),
                )
                if t == NS - 1:
                    # this half's z partial is complete: +bf1/8 and stage
                    # the reduce-exchange input immediately.
                    nc.vector.tensor_tensor(zsb[:, bh, :], zps[bh][:],
                                            bf1b[:], ALU.add)
                    nc.sync.dma_start(rsd[bh], zsb[:, bh, :])

            chunks = [(bh, s0) for bh in range(2) for s0 in range(0, NS, 4)]
            done = []
            for ci, (bh, s0) in enumerate(chunks):
                pm = psm.tile([128, 2 * B], f32, tag="psm")
                nc.tensor.matmul(pm[:], w2k_sb[:], r_sb[bh][:, s0:s0 + 4, :])
                e = ework.tile([128, 2 * B], bf16)
                nc.scalar.activation(e[:], pm[:], AF.Exp, bias=b2k_sb[:, 0:1])
                r = rwork.tile([128, 2 * B], bf16)
                nc.vector.tensor_scalar(r[:], pm[:], b2k_sb[0:128, 0:1], 0.0,
                                        ALU.add, ALU.max)
                nc.vector.tensor_scalar(e[:], e[:], 1.0, -1.0,
                                        ALU.min, ALU.add)
                eng = nc.gpsimd if ci % 2 == 0 else nc.vector
                eng.tensor_tensor(h2_sb[:, s0:s0 + 4, bh * 128:bh * 128 + 128],
                                  e[:], r[:], ALU.add)
                while done:
                    pbh, pt = done.pop(0)
                    fc1_mms(pbh, pt)
                done.extend((bh, t) for t in range(s0, s0 + 4))
            while done:
                pbh, pt = done.pop(0)
                fc1_mms(pbh, pt)
            # AllToAll (mesh, faster than RDH ReduceScatter) + local tree-add
            rs_out = dram.tile([NCORE, BPC, H], bf16)
            nc.gpsimd.collective_compute(
                "AllToAll", ALU.bypass,
                replica_groups=[list(range(NCORE))],
                ins=[rs_in.opt()], outs=[rs_out.opt()],
            )
            va = small.tile([BPC, NCORE, H], bf16)
            nc.sync.dma_start(va[:], rs_out[:].rearrange("k b h -> b k h"))
            t1 = small.tile([BPC, 4, H], f32)
            nc.vector.tensor_tensor(t1[:], va[:, 0:4, :], va[:, 4:8, :],
                                    ALU.add)
            t2 = small.tile([BPC, 2, H], f32)
            nc.vector.tensor_tensor(t2[:], t1[:, 0:2, :], t1[:, 2:4, :],
                                    ALU.add)
            zsum = small.tile([BPC, H], f32)
            nc.vector.tensor_tensor(zsum[:], t2[:, 0, :], t2[:, 1, :],
                                    ALU.add)

            # ---- relu, FC2 (+bf2), softmax --------------------------------
            zr = small.tile([BPC, H], f32)
            nc.scalar.activation(zr[:], zsum[:], AF.Relu)

            zrT = small.tile([128, HJ, BPC], bf16)
            for hj in range(HJ):
                pt = ps2.tile([128, BPC], f32, tag="ps2")
                nc.tensor.transpose(pt[:], zr[0:BPC, hj * 128:(hj + 1) * 128],
                                    id_sb[:])
                nc.scalar.copy(zrT[:, hj, :], pt[:])

            ps_o = ps1.tile([BPC, N_OUT], f32, tag="ps1")
            for hj in range(HJ):
                nc.tensor.matmul(ps_o[:], zrT[:, hj, :], wf2_sb[:, hj, :],
                                 start=(hj == 0), stop=False)
            nc.tensor.matmul(ps_o[:], ones[0:1, 0:BPC], bf2_sb[:],
                             start=False, stop=True)

            ex = small.tile([BPC, N_OUT], f32)
            sm = small.tile([BPC, 1], f32)
            nc.scalar.activation(ex[:], ps_o[:], AF.Exp, accum_out=sm[:])
            rc = small.tile([BPC, 1], f32)
            nc.vector.reciprocal(rc[:], sm[:])
            ob = small.tile([BPC, N_OUT], f32)
            nc.vector.tensor_scalar(ob[:], ex[:], rc[0:BPC, 0:1], None,
                                    ALU.mult)
            nc.sync.dma_start(out_d[:], ob[:])

    _install_wait_splitter(nc)
    return nc


_NC_CACHE = None


def _get_program():
    global _NC_CACHE
    if _NC_CACHE is None:
        _NC_CACHE = _build_program()
    return _NC_CACHE


# ---------------------------------------------------------------------------
def _prep_inputs(x, edge_row, edge_col, edge_val, W1, b1, W2, b2,
                 Wf1, bf1, Wf2, bf2):
    import ml_dtypes
    f = np.float32
    bf = ml_dtypes.bfloat16
    A = np.zeros((N, N), f)
    np.add.at(A, (np.asarray(edge_row), np.asarray(edge_col)),
              np.asarray(edge_val, f))
    AT = np.ascontiguousarray(A.T)                                  # [m, n]
    at = np.ascontiguousarray(
        AT.reshape(KN, P, N).transpose(1, 0, 2).reshape(P, KN * N)).astype(bf)

    XT = np.ascontiguousarray(np.asarray(x, f)[:, :, 0].T)          # [N, B]
    xt = np.ascontiguousarray(
        XT.reshape(KN, P, B).transpose(1, 0, 2).reshape(P, KN * B)).astype(bf)

    W1 = np.asarray(W1, f); b1 = np.asarray(b1, f)
    W2 = np.asarray(W2, f); b2 = np.asarray(b2, f)
    Wf1 = np.asarray(Wf1, f); bf1 = np.asarray(bf1, f)
    Wf2 = np.asarray(Wf2, f); bf2 = np.asarray(bf2, f)

    # mix weight: lhsT[(s4,c),(s4',c')] = delta(s4,s4') * W2[c,c']
    w2k = np.kron(np.eye(4, dtype=f), W2).astype(bf)                # [128,128]
    b2k = np.tile(b2, 4).reshape(128, 1).astype(f)

    # FC1: core k's K-chunk s holds flat rows (n=112k+ng*28+s)*C + c' at
    # partition p = ng*C + c'; rows for pad nodes (n >= 784) are zero.
    Wf1_pad = np.zeros((NPAD, C, H), f)
    Wf1_pad[:N] = Wf1.reshape(N, C, H)

    bf18 = (bf1 / NCORE).reshape(1, H).astype(f)
    wf2_l = np.ascontiguousarray(
        Wf2.reshape(HJ, 128, N_OUT).transpose(1, 0, 2).reshape(
            128, HJ * N_OUT)).astype(bf)
    bf2_l = bf2.reshape(1, N_OUT).astype(bf)
    idm = np.eye(BPC, dtype=f)

    in_maps = []
    for k in range(NCORE):
        wb = np.concatenate([W1[0, k * CPC:(k + 1) * CPC],
                             b1[k * CPC:(k + 1) * CPC]]).reshape(1, 2 * CPC)
        # [NG, NS, C, H] -> chunk s, partition (ng, c')
        wk = Wf1_pad[k * P:(k + 1) * P].reshape(NG, NS, C, H)
        wf1_l = np.ascontiguousarray(
            wk.transpose(1, 0, 2, 3).reshape(NS * 128, H)).astype(bf)
        in_maps.append({
            "at": at, "xt": xt, "wf1": wf1_l,
            "wb": np.ascontiguousarray(wb.astype(f)),
            "w2k": w2k, "b2k": b2k,
            "bf18": bf18, "wf2": wf2_l, "bf2": bf2_l, "idm": idm,
        })
    return in_maps


def kernel(x, edge_row, edge_col, edge_val, W1, b1, W2, b2,
           Wf1, bf1, Wf2, bf2, **kw):
    nc = _get_program()
    in_maps = _prep_inputs(x, edge_row, edge_col, edge_val, W1, b1, W2, b2,
                           Wf1, bf1, Wf2, bf2)
    res = run_bass_kernel_spmd(nc, in_maps, list(range(NCORE)), **kw)
    out = np.concatenate([res.results[k]["out"] for k in range(NCORE)], axis=0)
    if kw.get("trace"):
        kernel.last_exec_time_ns = res.exec_time_ns
    return out.astype(np.float32)
